# revision 1
# baseline (speedup 1.0000x reference)
# GCN (2-layer GCNConv + BatchNorm + ReLU + global mean pool) on 8 TRN2 NeuronCores.
#
# Math (reference):
#   deg[v]  = in-degree incl. self-loop;  dinv = deg^-1/2
#   layer(x, W, b): h = D^-1/2 (A+I) D^-1/2 (x W) + b
#                 = (dinv * (sum_{e: dst=v} xs[src_e] + xs[v])) W + b,  xs = dinv*x
#   h1 = relu(batchnorm(layer1));  h2 = layer2(h1);  out = segment_mean(h2, batch)
#
# Sharding: core k owns nodes [k*SL, (k+1)*SL) and all edges whose dst falls in
# that range.  Per layer: all-gather the (dinv-scaled) node table (1.6 MB/core
# shard), then per-core dma_gather x[src] rows from the table and
# dma_scatter_add them into an SBUF-resident accumulator (the parity-split
# CCE layout, so the += happens in the SDMA datapath without HBM RMW).
#
# Hardware constraints shape the edge schedule (all verified empirically):
#   * dma_scatter_add races (loses updates) for duplicate indices within one
#     instruction -> each destination gets T=5 accumulator rows
#     (dst + t*SLP); edges are split into "super-rounds" where round s holds
#     occurrences [s*T, (s+1)*T) of every destination, so rows are unique per
#     instruction.  The copies are merged afterwards with 8 strided DVE adds.
#   * the SWDGE descriptor ring holds ~512 descriptors -> instructions are
#     capped at 7680 slots (and single_packet=False is required).
#   * gather indices are signed int16 -> edges are bucketed by src row
#     (< 32768 vs >=) and gathered from two table base offsets; within each
#     instruction segment edges are sorted by src for HBM row locality.
# BatchNorm stats via an accumulated A^T[A|1] matmul + algebraic reduction
# (mean/var of A@W1+b1 from A^T A, A^T 1) + one tiny all-reduce.  Pooling and
# the outer D^-1/2 are folded into matmuls with a host-built (P * dinv)
# matrix; the final [64,64] partial is all-reduced.
#
# Host-side preprocessing uses only index data (edge_index, batch): degree
# computation, edge partitioning/sorting/round assignment, pooling matrix.
# Feature data is never touched on the host.

import os

import numpy as np

N_NODES = 50000
N_EDGES = 800000
D = 64
NCORES = 8
NUM_GRAPHS = 64
BN_EPS = 1e-5
SPLIT = 32768  # int16 gather index limit


class Cfg:
    def __init__(self, n, sl):
        self.N = n                    # total nodes
        self.SL = sl                  # owned nodes per core
        self.SLP = ((sl + 127) // 128) * 128   # padded slice rows
        assert self.SL < self.SLP, "need a pad row in the accumulator slice"
        self.NT = self.SLP // 128     # 128-row node tiles per slice
        self.NG = NCORES * self.SLP   # padded global table rows
        assert self.NG < 2 * SPLIT
        self.T = 5                    # accumulator copies per dst
        assert self.T * self.SLP < SPLIT
        self.CAP = 7680               # max slots per instruction (SWDGE ring: ~512 descs)
        # super-round sizes (lo/hi gather slots), filled by prepare_inputs
        self.a = []                   # lo-bucket slots per instruction (mult of 128)
        self.b = []                   # hi-bucket slots per instruction (mult of 128)
        self.pair = []                # accumulator pair (0/1) per instruction


LAST_EXEC_TIME_NS = None
_NC_CACHE = {}
_LAST_IN_MAPS = None


def build(cfg):
    import concourse.mybir as mybir
    import concourse.tile as tile
    from concourse import bacc
    from concourse.masks import make_identity

    f32 = mybir.dt.float32
    i16 = mybir.dt.int16
    SL, SLP, NT, NG = cfg.SL, cfg.SLP, cfg.NT, cfg.NG
    T = cfg.T
    PAIR_T = (2, 2, 1)                # copies per accumulator pair
    assert sum(PAIR_T) == T
    NGRPS = [(t * NT + 1) // 2 for t in PAIR_T]
    G0_OWN = (NT + 1) // 2            # first non-copy0 group in own_a
    G0_PEER = NT // 2                 # first non-copy0 group in peer_a
    NN = float(cfg.N)
    RG = [list(range(NCORES))]
    R = len(cfg.a)
    ssz = [cfg.a[c] + cfg.b[c] for c in range(R)]
    SMAX = max(ssz)
    tot_lo = sum(cfg.a)
    tot_hi = sum(cfg.b)
    tot_s = sum(ssz)

    nc = bacc.Bacc(
        "TRN2", target_bir_lowering=False, debug=False, num_devices=NCORES
    )

    # --- external inputs (per-core values supplied via in_maps) ---
    xsl = nc.declare_dram_parameter("xsl", [SLP, D], f32, isOutput=False)
    dinv_in = nc.declare_dram_parameter("dinv_in", [128, NT], f32, isOutput=False)
    glo_d = (nc.declare_dram_parameter("glo", [128, tot_lo // 16], i16, isOutput=False)
             if tot_lo else None)
    ghi_d = (nc.declare_dram_parameter("ghi", [128, tot_hi // 16], i16, isOutput=False)
             if tot_hi else None)
    sct_d = nc.declare_dram_parameter("sct", [128, tot_s // 16], i16, isOutput=False)
    pt_d = nc.declare_dram_parameter("pt", [SLP, NUM_GRAPHS], f32, isOutput=False)
    p1_d = nc.declare_dram_parameter("p1", [1, NUM_GRAPHS], f32, isOutput=False)
    w1_d = nc.declare_dram_parameter("w1", [D, D], f32, isOutput=False)
    b1_d = nc.declare_dram_parameter("b1", [D, 1], f32, isOutput=False)
    ga_d = nc.declare_dram_parameter("ga", [D, 1], f32, isOutput=False)
    be_d = nc.declare_dram_parameter("be", [D, 1], f32, isOutput=False)
    w2_d = nc.declare_dram_parameter("w2", [D, D], f32, isOutput=False)
    b2_d = nc.declare_dram_parameter("b2", [1, D], f32, isOutput=False)
    out_d = nc.declare_dram_parameter("out", [NUM_GRAPHS, D], f32, isOutput=True)

    # --- internal DRAM ---
    ag1_in = nc.dram_tensor("ag1_in", [SLP, D], f32)
    table1 = nc.dram_tensor("table1", [NG, D], f32, addr_space="Shared")
    ag2_in = nc.dram_tensor("ag2_in", [SLP, D], f32)
    table2 = nc.dram_tensor("table2", [NG, D], f32, addr_space="Shared")
    ars_in = nc.dram_tensor("ars_in", [D, D + 1], f32)
    ars_out = nc.dram_tensor("ars_out", [D, D + 1], f32, addr_space="Shared")
    aro_in = nc.dram_tensor("aro_in", [NUM_GRAPHS, D], f32)
    aro_out = nc.dram_tensor("aro_out", [NUM_GRAPHS, D], f32, addr_space="Shared")

    with tile.TileContext(nc) as tc:
        with (
            tc.tile_pool(name="const", bufs=1) as const,
            tc.tile_pool(name="persist", bufs=1) as persist,
            tc.tile_pool(name="work", bufs=3) as work,
            tc.tile_pool(name="msgp", bufs=3) as msgp,
            tc.tile_pool(name="spsum", bufs=1, space="PSUM") as spsum,
            tc.tile_pool(name="wpsum", bufs=2, space="PSUM") as wpsum,
        ):
            # --- constants into SBUF ---
            w1s = const.tile([D, D], f32)
            nc.sync.dma_start(out=w1s[:], in_=w1_d[:, :])
            w2s = const.tile([D, D], f32)
            nc.sync.dma_start(out=w2s[:], in_=w2_d[:, :])
            b1c = const.tile([D, 1], f32)
            nc.sync.dma_start(out=b1c[:], in_=b1_d[:, :])
            gac = const.tile([D, 1], f32)
            nc.sync.dma_start(out=gac[:], in_=ga_d[:, :])
            bec = const.tile([D, 1], f32)
            nc.sync.dma_start(out=bec[:], in_=be_d[:, :])
            b2r = const.tile([1, D], f32)
            nc.sync.dma_start(out=b2r[:], in_=b2_d[:, :])
            p1s = const.tile([1, NUM_GRAPHS], f32)
            nc.sync.dma_start(out=p1s[:], in_=p1_d[:, :])
            dinvs = const.tile([128, NT], f32)
            nc.sync.dma_start(out=dinvs[:], in_=dinv_in[:, :])
            ident = const.tile([128, 128], f32)
            make_identity(nc, ident[:])
            ones64 = const.tile([D, 1], f32)
            nc.vector.memset(ones64[:], 1.0)
            epsc = const.tile([D, 1], f32)
            nc.vector.memset(epsc[:], BN_EPS)

            ablate = os.environ.get("GNN_ABLATE", "")

            # --- persistent edge-index tiles (shared by both layers) ---
            if "noidx" not in ablate:
                if tot_lo:
                    glo_t = persist.tile([128, tot_lo // 16], i16, name="glo_t")
                    nc.sync.dma_start(out=glo_t[:], in_=glo_d[:, :])
                if tot_hi:
                    ghi_t = persist.tile([128, tot_hi // 16], i16, name="ghi_t")
                    nc.sync.dma_start(out=ghi_t[:], in_=ghi_d[:, :])
                sct_t = persist.tile([128, tot_s // 16], i16, name="sct_t")
                nc.sync.dma_start(out=sct_t[:], in_=sct_d[:, :])
            glo_off, ghi_off, sct_off = [], [], []
            olo = ohi = osc = 0
            for c in range(R):
                glo_off.append(olo); ghi_off.append(ohi); sct_off.append(osc)
                olo += cfg.a[c] // 16
                ohi += cfg.b[c] // 16
                osc += ssz[c] // 16

            # --- SBUF accumulators (parity-split scatter layout, T copies) ---
            # acc row l (< T*SLP) -> partition l%128, slot s=l>>7;
            # even s in own[:, s>>1], odd s in peer[:, s>>1].
            # copy t of node tile b lives at slot t*NT + b; copy 0 is merged
            # into by merge_copies() after each scatter phase.
            own = persist.tile([128, NGRPS[0], D], f32, name="own")
            peer = persist.tile([128, NGRPS[0], D], f32, name="peer")
            own_b = persist.tile([128, NGRPS[1], D], f32, name="own_b")
            peer_b = persist.tile([128, NGRPS[1], D], f32, name="peer_b")
            own_c = persist.tile([128, NGRPS[2], D], f32, name="own_c")
            peer_c = persist.tile([128, NGRPS[2], D], f32, name="peer_c")
            PAIRS = [(own, peer), (own_b, peer_b), (own_c, peer_c)]

            def acc_slice(b):
                t = (own, peer)[b % 2]
                return t[:, b // 2, :]

            def zero_copies():
                nc.vector.memset(own[:, G0_OWN:, :], 0.0)
                nc.vector.memset(peer[:, G0_PEER:, :], 0.0)
                for o, p_ in PAIRS[1:]:
                    nc.vector.memset(o[:], 0.0)
                    nc.vector.memset(p_[:], 0.0)

            def merge_copies():
                # all non-(pairA,copy0) copies into pair-A copy 0
                jobs = [((own, peer), t * NT) for t in range(1, PAIR_T[0])]
                for pi in range(1, len(PAIRS)):
                    jobs += [(PAIRS[pi], t * NT) for t in range(PAIR_T[pi])]
                for src_pair, S in jobs:
                    for p in (0, 1):
                        ng = (NT - p + 1) // 2
                        sp = (S + p) % 2
                        g0 = (S + p) // 2
                        dst_t = (own, peer)[p]
                        src_t = src_pair[sp]
                        nc.vector.tensor_tensor(
                            out=dst_t[:, 0:ng, :], in0=dst_t[:, 0:ng, :],
                            in1=src_t[:, g0 : g0 + ng, :],
                            op=mybir.AluOpType.add,
                        )

            # --- phase A: xs = dinv * x  -> acc1 init (self-loop) + AG input ---
            # big node-major view: node b*128+p -> xt_big[p, b, :]
            def acc_strided(par):
                # [128, ngroups, D] view of copy-0 tiles for b%2==par
                ng = (NT - par + 1) // 2
                return (own, peer)[par][:, 0:ng, :], ng

            def dinv_b(par, ng):
                # [128, ng, 1] broadcast AP of dinv for b = 2g+par
                return dinvs[:, par : par + 2 * ng - 1 : 2].rearrange(
                    "p (g o) -> p g o", o=1).to_broadcast([128, ng, D])

            def ag_write(dram, par, ng):
                src_t = (own, peer)[par][:, 0:ng, :]
                dst_ap = dram.rearrange("(g p) d -> p g d", p=128)[:, par : par + 2 * ng - 1 : 2, :]
                nc.sync.dma_start(out=dst_ap, in_=src_t)

            if "noA" not in ablate:
                xt_big = work.tile([128, NT, D], f32, tag="xtb", name="xt_big", bufs=1)
                nc.sync.dma_start(
                    out=xt_big[:], in_=xsl[:, :].rearrange("(g p) d -> p g d", p=128)
                )
                for par in (0, 1):
                    t, ng = acc_strided(par)
                    nc.vector.tensor_tensor(
                        out=t, in0=xt_big[:, par : par + 2 * ng - 1 : 2, :],
                        in1=dinv_b(par, ng), op=mybir.AluOpType.mult,
                    )
                    ag_write(ag1_in[:, :], par, ng)
            zero_copies()

            # --- phase B: all-gather layer-1 table ---
            def do_cc(kind, op, ins_ap, outs_ap):
                if "nocc" in ablate:
                    nc.sync.dma_start(out=outs_ap[0 : ins_ap.shape[0], :], in_=ins_ap)
                    return
                nc.gpsimd.collective_compute(
                    kind, op, replica_groups=RG, ins=[ins_ap], outs=[outs_ap],
                )

            do_cc("AllGather", mybir.AluOpType.bypass, ag1_in[:, :], table1[:, :])

            def edge_phase(table):
                if "noedge" in ablate:
                    return
                for c in range(R):
                    ac, bc, sc = cfg.a[c], cfg.b[c], ssz[c]
                    msg = msgp.tile([128, SMAX // 128, D], f32, tag="msg", name="msg")
                    if ac and "nogather" not in ablate:
                        nc.gpsimd.dma_gather(
                            out_ap=msg[:, : ac // 128, :],
                            in_ap=table[0 : min(SPLIT, NG), :],
                            idxs_ap=glo_t[:, glo_off[c] : glo_off[c] + ac // 16],
                            num_idxs=ac, num_idxs_reg=ac, elem_size=D,
                            single_packet=False, queue_num=0,
                        )
                    if bc and "nogather" not in ablate:
                        nc.gpsimd.dma_gather(
                            out_ap=msg[:, ac // 128 : sc // 128, :],
                            in_ap=table[SPLIT:NG, :],
                            idxs_ap=ghi_t[:, ghi_off[c] : ghi_off[c] + bc // 16],
                            num_idxs=bc, num_idxs_reg=bc, elem_size=D,
                            single_packet=False, queue_num=0,
                        )
                    if "noscatter" in ablate:
                        continue
                    t_own, t_peer = PAIRS[cfg.pair[c]]
                    nc.gpsimd.dma_scatter_add(
                        t_own[:], msg[:, : sc // 128, :],
                        sct_t[:, sct_off[c] : sct_off[c] + sc // 16],
                        sc, sc, D,
                        sbuf_tokens_per_rank=128, parity_reg=0,
                        out_ap_other=t_peer[:],
                        single_packet=False, queue_num=0,
                    )

            # --- phase C: layer-1 edges ---
            edge_phase(table1)
            merge_copies()

            # --- phase D: layer-1 dense compute (transposed) + BN stats ---
            import concourse.mybir as mb

            stats_ps = spsum.tile([D, D + 1], f32, name="stats_ps")
            aggs = persist.tile([128, NT, D + 1], f32, name="aggs")
            nc.vector.memset(aggs[:, :, D : D + 1], 1.0)
            for par in (0, 1):
                t, ng = acc_strided(par)
                nc.vector.tensor_tensor(
                    out=aggs[:, par : par + 2 * ng - 1 : 2, :D],
                    in0=t, in1=dinv_b(par, ng), op=mybir.AluOpType.mult,
                )
            hT_big = persist.tile([D, NT * 128], f32, name="hT_big")
            ND = NT if "noD" not in ablate else 1
            for b0 in range(0, ND, 4):
                bn = min(4, ND - b0)
                tp_ps = wpsum.tile([D, 512], f32, tag="ps_a", name="tp_ps")
                for j in range(bn):
                    b = b0 + j
                    t_in = aggs[:, b, :]
                    nc.tensor.matmul(
                        out=stats_ps[:], lhsT=t_in[:, :D], rhs=t_in[:, : D + 1],
                        start=(b == 0), stop=(b == ND - 1),
                    )
                    nc.tensor.transpose(
                        out=tp_ps[:, j * 128 : (j + 1) * 128],
                        in_=t_in[:, :D], identity=ident[:],
                    )
                aggsT = work.tile([D, 512], f32, tag="aggsT", name="aggsT", bufs=2)
                nc.vector.tensor_copy(out=aggsT[:, : bn * 128], in_=tp_ps[:, : bn * 128])
                hT_ps = wpsum.tile([D, 512], f32, tag="ps_b", name="hT_ps")
                nc.tensor.matmul(
                    out=hT_ps[:, : bn * 128], lhsT=w1s[:], rhs=aggsT[:, : bn * 128],
                    start=True, stop=True,
                )
                nc.vector.tensor_copy(
                    out=hT_big[:, b0 * 128 : (b0 + bn) * 128],
                    in_=hT_ps[:, : bn * 128],
                )

            # --- phase E: BN stats all-reduce + scalar algebra ---
            stats_sb = persist.tile([D, D + 1], f32, name="stats_sb")
            nc.vector.tensor_copy(out=stats_sb[:], in_=stats_ps[:])
            nc.sync.dma_start(out=ars_in[:, :], in_=stats_sb[:])
            do_cc("AllReduce", mybir.AluOpType.add, ars_in[:, :], ars_out[:, :])
            st = persist.tile([D, D + 1], f32, name="st")
            nc.sync.dma_start(out=st[:], in_=ars_out[:, :])

            q_ps = wpsum.tile([D, 1], f32, tag="ps_a", name="q_ps")
            nc.tensor.matmul(out=q_ps[:], lhsT=w1s[:], rhs=st[:, D : D + 1], start=True, stop=True)
            mu = persist.tile([D, 1], f32, name="mu")
            nc.vector.tensor_scalar(
                out=mu[:], in0=q_ps[:], scalar1=1.0 / NN, scalar2=b1c[:],
                op0=mybir.AluOpType.mult, op1=mybir.AluOpType.add,
            )
            t1_ps = wpsum.tile([D, D], f32, tag="ps_b", name="t1_ps")
            nc.tensor.matmul(out=t1_ps[:], lhsT=st[:, :D], rhs=w1s[:], start=True, stop=True)
            m_sb = work.tile([D, D], f32, tag="m_sb", name="m_sb")
            nc.vector.tensor_tensor(out=m_sb[:], in0=w1s[:], in1=t1_ps[:], op=mybir.AluOpType.mult)
            d_ps = wpsum.tile([D, 1], f32, tag="ps_b", name="d_ps")
            nc.tensor.matmul(out=d_ps[:], lhsT=m_sb[:], rhs=ones64[:], start=True, stop=True)

            var = persist.tile([D, 1], f32, name="var")
            nc.vector.tensor_scalar_mul(out=var[:], in0=d_ps[:], scalar1=1.0 / NN)
            t2 = work.tile([D, 1], f32, tag="t2", name="t2")
            nc.vector.tensor_scalar_mul(out=t2[:], in0=q_ps[:], scalar1=2.0 / NN)
            nc.vector.tensor_tensor(out=t2[:], in0=t2[:], in1=b1c[:], op=mybir.AluOpType.mult)
            nc.vector.tensor_tensor(out=var[:], in0=var[:], in1=t2[:], op=mybir.AluOpType.add)
            t3 = work.tile([D, 1], f32, tag="t3", name="t3")
            nc.vector.tensor_tensor(out=t3[:], in0=b1c[:], in1=b1c[:], op=mybir.AluOpType.mult)
            nc.vector.tensor_tensor(out=var[:], in0=var[:], in1=t3[:], op=mybir.AluOpType.add)
            t4 = work.tile([D, 1], f32, tag="t4", name="t4")
            nc.vector.tensor_tensor(out=t4[:], in0=mu[:], in1=mu[:], op=mybir.AluOpType.mult)
            nc.vector.tensor_tensor(out=var[:], in0=var[:], in1=t4[:], op=mybir.AluOpType.subtract)

            sd = work.tile([D, 1], f32, tag="sd", name="sd")
            nc.scalar.activation(sd[:], var[:], mb.ActivationFunctionType.Sqrt, bias=epsc[:])
            rstd = work.tile([D, 1], f32, tag="rstd", name="rstd")
            nc.vector.reciprocal(out=rstd[:], in_=sd[:])
            a_sb = persist.tile([D, 1], f32, name="a_sb")
            nc.vector.tensor_tensor(out=a_sb[:], in0=gac[:], in1=rstd[:], op=mybir.AluOpType.mult)
            c_sb = persist.tile([D, 1], f32, name="c_sb")
            t5 = work.tile([D, 1], f32, tag="t5", name="t5")
            nc.vector.tensor_tensor(out=t5[:], in0=mu[:], in1=a_sb[:], op=mybir.AluOpType.mult)
            nc.vector.tensor_tensor(out=c_sb[:], in0=bec[:], in1=t5[:], op=mybir.AluOpType.subtract)
            # hT tiles exclude the b1 bias; fold it into the BN offset:
            # relu(a*(h+b1) + c) = relu(a*h + (c + a*b1))
            t6 = work.tile([D, 1], f32, tag="t6", name="t6")
            nc.vector.tensor_tensor(out=t6[:], in0=a_sb[:], in1=b1c[:], op=mybir.AluOpType.mult)
            nc.vector.tensor_tensor(out=c_sb[:], in0=c_sb[:], in1=t6[:], op=mybir.AluOpType.add)

            # --- phase F: BN+ReLU, transpose back, dinv fold -> acc2 init + AG ---
            NF = NT if "noF" not in ablate else 0
            for b0 in range(0, NF, 4):
                bn = min(4, NF - b0)
                h1T = work.tile([D, 512], f32, tag="h1T", name="h1T", bufs=2)
                nc.scalar.activation(
                    h1T[:, : bn * 128],
                    hT_big[:, b0 * 128 : (b0 + bn) * 128],
                    mb.ActivationFunctionType.Relu,
                    bias=c_sb[:], scale=a_sb[:],
                )
                for j in range(bn):
                    b = b0 + j
                    nm_ps = wpsum.tile([128, D], f32, tag="ps_a", name="nm_ps")
                    nc.tensor.transpose(
                        out=nm_ps[:], in_=h1T[:, j * 128 : (j + 1) * 128],
                        identity=ident[:D, :D],
                    )
                    dst = acc_slice(b)
                    nc.vector.tensor_scalar_mul(out=dst, in0=nm_ps[:], scalar1=dinvs[:, b : b + 1])
            if "noF" not in ablate:
                for par in (0, 1):
                    _, ng = acc_strided(par)
                    ag_write(ag2_in[:, :], par, ng)
            zero_copies()

            # --- phase G: all-gather layer-2 table ---
            do_cc("AllGather", mybir.AluOpType.bypass, ag2_in[:, :], table2[:, :])

            # --- phase H: layer-2 edges ---
            edge_phase(table2)
            merge_copies()

            # --- phase I: pooling matmul accumulate: poolT = acc2^T @ P'^T ---
            poolT_ps = spsum.tile([D, NUM_GRAPHS], f32, name="poolT_ps")
            pt_big = persist.tile([128, NT, NUM_GRAPHS], f32, name="pt_big")
            nc.sync.dma_start(
                out=pt_big[:], in_=pt_d[:, :].rearrange("(g p) d -> p g d", p=128)
            )
            for b in range(NT if "noI" not in ablate else 1):
                nc.tensor.matmul(
                    out=poolT_ps[:], lhsT=acc_slice(b), rhs=pt_big[:, b, :],
                    start=(b == 0), stop=(b == NT - 1) or ("noI" in ablate),
                )

            # --- phase J: out = pool @ W2 + p1^T b2 ; all-reduce ---
            poolT_sb = persist.tile([D, NUM_GRAPHS], f32, name="poolT_sb")
            nc.vector.tensor_copy(out=poolT_sb[:], in_=poolT_ps[:])
            out_ps = wpsum.tile([NUM_GRAPHS, D], f32, tag="ps_a", name="out_ps")
            nc.tensor.matmul(out=out_ps[:], lhsT=poolT_sb[:], rhs=w2s[:], start=True, stop=False)
            nc.tensor.matmul(out=out_ps[:], lhsT=p1s[:], rhs=b2r[:], start=False, stop=True)
            out_sb = persist.tile([NUM_GRAPHS, D], f32, name="out_sb")
            nc.vector.tensor_copy(out=out_sb[:], in_=out_ps[:])
            nc.sync.dma_start(out=aro_in[:, :], in_=out_sb[:])
            do_cc("AllReduce", mybir.AluOpType.add, aro_in[:, :], aro_out[:, :])
            nc.sync.dma_start(out=out_d[:, :], in_=aro_out[:, :])

    nc.compile()
    return nc


def _wrap16(v, n):
    """idx j at [j%16, j//16], replicated to 128 partitions (8 Q7 cores)."""
    assert v.shape[0] == n and n % 16 == 0
    t = v.astype(np.int16).reshape(n // 16, 16).T
    return np.tile(t, (8, 1))


def _super_rounds(cfg, ed, eg):
    """Split one core's (dst-sorted) edges into super-rounds: round s holds
    occurrences [s*T, (s+1)*T) of each dst, scattered to accumulator row
    dst + (occ - s*T)*SLP (unique rows within a round).  Each round is
    bucketed by src < SPLIT.  Returns per-round (lo_src, lo_row, hi_src,
    hi_row) arrays."""
    T, SLP = cfg.T, cfg.SLP
    nk = ed.shape[0]
    if nk == 0:
        return []
    change = np.r_[True, ed[1:] != ed[:-1]]
    starts = np.flatnonzero(change)
    gid = np.cumsum(change) - 1
    occ = np.arange(nk) - starts[gid]
    copy = occ % T
    pair = copy // 2                       # copies (0,1)->0, (2,3)->1, (4)->2
    row = ed + (copy % 2) * SLP
    sr = occ // T
    out = []
    for s in range(int(sr.max()) + 1):
        for pr in (0, 1, 2):
            m = (sr == s) & (pair == pr)
            g, r = eg[m], row[m]
            lo = g < SPLIT
            out.append((g[lo], r[lo], g[~lo] - SPLIT, r[~lo]))
    return out


def prepare_inputs(cfg, x, edge_index, batch, W1, b1, gamma, beta, W2, b2):
    """Host-side index preprocessing + per-core input maps.  Also fills
    cfg.a / cfg.b (shared per-round slot counts)."""
    SL, SLP = cfg.SL, cfg.SLP
    n = cfg.N

    x = np.ascontiguousarray(np.asarray(x, dtype=np.float32))
    src = np.asarray(edge_index[0], dtype=np.int64)
    dst = np.asarray(edge_index[1], dtype=np.int64)
    batch = np.asarray(batch, dtype=np.int64)
    W1 = np.asarray(W1, dtype=np.float32)
    b1 = np.asarray(b1, dtype=np.float32)
    gamma = np.asarray(gamma, dtype=np.float32)
    beta = np.asarray(beta, dtype=np.float32)
    W2 = np.asarray(W2, dtype=np.float32)
    b2 = np.asarray(b2, dtype=np.float32)

    deg = np.bincount(dst, minlength=n).astype(np.float32) + 1.0  # + self-loop
    dinv = (1.0 / np.sqrt(deg)).astype(np.float32)

    owner = dst // SL
    dst_local = (dst - owner * SL).astype(np.int64)
    gsrc = ((src // SL) * SLP + (src % SL)).astype(np.int64)

    cnt = np.bincount(batch, minlength=NUM_GRAPHS).astype(np.float32)
    w_graph = 1.0 / np.maximum(cnt, 1.0)

    per_core = []
    for k in range(NCORES):
        sel = owner == k
        ed = dst_local[sel]
        eg = gsrc[sel]
        order = np.argsort(ed, kind="stable")
        per_core.append(_super_rounds(cfg, ed[order], eg[order]))

    NSR = max(len(r) for r in per_core)
    up = lambda v: ((v + 127) // 128) * 128 if v else 0
    # common padded lo/hi sizes per super-round
    A = [up(max((len(rc[s][0]) if s < len(rc) else 0) for rc in per_core))
         for s in range(NSR)]
    B = [up(max((len(rc[s][2]) if s < len(rc) else 0) for rc in per_core))
         for s in range(NSR)]
    for s in range(NSR):
        if A[s] == 0 and B[s] == 0:
            A[s] = 128
    # split each super-round's common [lo | hi] slot layout into
    # instructions of <= CAP slots; record per-instruction lo/hi sizes and
    # the originating super-round + slot offsets for host data emission
    cfg.a, cfg.b, cfg.pair = [], [], []
    pieces = []  # (group, lo_start, hi_start) per instruction
    for s in range(NSR):
        tot = A[s] + B[s]
        pos = 0
        while pos < tot:
            en = min(pos + cfg.CAP, tot)
            ai = max(0, min(en, A[s]) - pos)
            bi = max(0, en - max(pos, A[s]))
            cfg.a.append(ai)
            cfg.b.append(bi)
            cfg.pair.append(s % 3)
            pieces.append((s, pos, max(0, pos - A[s]) if pos >= A[s] else 0))
            pos = en

    in_maps = []
    for k in range(NCORES):
        rc = per_core[k]
        # per-super-round padded arrays in the common layout
        sr_gl, sr_gh, sr_sl, sr_sh = [], [], [], []
        for s in range(NSR):
            ls, ld, hs, hd = (rc[s] if s < len(rc)
                              else (np.zeros(0, np.int64),) * 4)
            gl = np.zeros(A[s], dtype=np.int64)
            gl[: len(ls)] = ls
            sc_lo = np.full(A[s], SL, dtype=np.int64)
            sc_lo[: len(ld)] = ld
            gh = np.zeros(B[s], dtype=np.int64)
            gh[: len(hs)] = hs
            sc_hi = np.full(B[s], SL, dtype=np.int64)
            sc_hi[: len(hd)] = hd
            sr_gl.append(gl); sr_gh.append(gh)
            sr_sl.append(sc_lo); sr_sh.append(sc_hi)
        def _src_sorted(g, sc):
            # sort real (non-pad) slots by gather row for HBM locality;
            # pads (scatter row == SL with gather row 0) stay at the tail
            real = sc != SL
            nreal = int(real.sum())
            g2, sc2 = g.copy(), sc.copy()
            order = np.argsort(g[:nreal], kind="stable")
            g2[:nreal] = g[:nreal][order]
            sc2[:nreal] = sc[:nreal][order]
            return g2, sc2

        glo_parts, ghi_parts, sct_parts = [], [], []
        for i, (s, pos, _) in enumerate(pieces):
            ac, bc = cfg.a[i], cfg.b[i]
            if ac:
                lo0 = pos
                gl, sct_lo = _src_sorted(sr_gl[s][lo0 : lo0 + ac],
                                         sr_sl[s][lo0 : lo0 + ac])
                glo_parts.append(_wrap16(gl, ac))
            else:
                sct_lo = np.zeros(0, np.int64)
            if bc:
                hi0 = max(0, pos - A[s])
                gh, sct_hi = _src_sorted(sr_gh[s][hi0 : hi0 + bc],
                                         sr_sh[s][hi0 : hi0 + bc])
                ghi_parts.append(_wrap16(gh, bc))
            else:
                sct_hi = np.zeros(0, np.int64)
            sct_parts.append(_wrap16(np.concatenate([sct_lo, sct_hi]), ac + bc))
        glo = (np.concatenate(glo_parts, axis=1) if glo_parts
               else np.zeros((128, 0), np.int16))
        ghi = (np.concatenate(ghi_parts, axis=1) if ghi_parts
               else np.zeros((128, 0), np.int16))
        sct = np.concatenate(sct_parts, axis=1)

        lo, hi = k * SL, min((k + 1) * SL, n)
        nsl = hi - lo
        xsl = np.zeros((SLP, D), dtype=np.float32)
        xsl[:nsl] = x[lo:hi]
        dsl = np.zeros(SLP, dtype=np.float32)
        dsl[:nsl] = dinv[lo:hi]
        dinv_in = dsl.reshape(cfg.NT, 128).T.copy()

        pt = np.zeros((SLP, NUM_GRAPHS), dtype=np.float32)
        bsl = batch[lo:hi]
        pt[np.arange(nsl), bsl] = w_graph[bsl] * dinv[lo:hi]
        p1 = np.zeros((1, NUM_GRAPHS), dtype=np.float32)
        np.add.at(p1[0], bsl, w_graph[bsl])

        im = {
                "xsl": xsl,
                "dinv_in": dinv_in,
                "sct": np.ascontiguousarray(sct),
                "pt": pt,
                "p1": p1,
                "w1": W1,
                "b1": b1.reshape(D, 1),
                "ga": gamma.reshape(D, 1),
                "be": beta.reshape(D, 1),
                "w2": W2,
                "b2": b2.reshape(1, D),
        }
        if glo.shape[1]:
            im["glo"] = np.ascontiguousarray(glo)
        if ghi.shape[1]:
            im["ghi"] = np.ascontiguousarray(ghi)
        in_maps.append(im)
    return in_maps


def kernel(x, edge_index, batch, W1, b1, gamma, beta, W2, b2):
    global LAST_EXEC_TIME_NS
    from concourse.bass_utils import run_bass_kernel_spmd

    cfg = Cfg(N_NODES, N_NODES // NCORES)
    in_maps = prepare_inputs(cfg, x, edge_index, batch, W1, b1, gamma, beta, W2, b2)

    key = (cfg.N, cfg.SL, tuple(cfg.a), tuple(cfg.b))
    if key not in _NC_CACHE:
        _NC_CACHE[key] = build(cfg)
    nc = _NC_CACHE[key]
    global _LAST_IN_MAPS
    _LAST_IN_MAPS = in_maps

    trace = bool(int(os.environ.get("BASS_GNN_TRACE", "0")))
    if trace:
        try:
            res = run_bass_kernel_spmd(nc, in_maps, list(range(NCORES)), trace=True)
        except Exception:
            res = run_bass_kernel_spmd(nc, in_maps, list(range(NCORES)), trace=False)
    else:
        res = run_bass_kernel_spmd(nc, in_maps, list(range(NCORES)), trace=False)
    LAST_EXEC_TIME_NS = res.exec_time_ns
    return np.asarray(res.results[0]["out"], dtype=np.float32)


def modeled_time_ns(x=None, edge_index=None, **kw):
    """Cost-model execution time (MultiCoreSim, mocked collectives) for the
    current cached program; used when NTFF tracing is unavailable."""
    if not _NC_CACHE:
        return None
    nc = next(iter(_NC_CACHE.values()))
    ins = _LAST_IN_MAPS
    if ins is None:
        return None
    from concourse.bass_interp import MultiCoreSim

    sim = MultiCoreSim(nc, 2, debug_mock_collectives_without_correctness=True)
    for i, core in sim.cores.items():
        for name, val in ins[i].items():
            core.tensor(name)[:] = val
    sim.simulate()
    return int(sim.global_time)



# revision 7
# speedup vs baseline: 2.9664x; 2.9664x over previous
# GCN (2-layer GCNConv + BatchNorm + ReLU + global mean pool) on 8 TRN2 NeuronCores.
#
# Math (reference):
#   deg[v]  = in-degree incl. self-loop;  dinv = deg^-1/2
#   layer(x, W, b): h = D^-1/2 (A+I) D^-1/2 (x W) + b
#   h1 = relu(batchnorm(layer1));  h2 = layer2(h1);  out = segment_mean(h2, batch)
#
# Sharding (v2 — source-partitioned edges + ReduceScatter):
#   Core k owns nodes [k*SL, (k+1)*SL) and all edges whose SRC falls in that
#   range (plus its own self-loop edges).  Layer 1:
#     * xs = dinv * x (own slice) -> local gather table (DRAM), so the edge
#       gather needs NO collective at all.
#     * per-edge: dma_gather xs[src] rows from the local table, then
#       dma_scatter_add into a full-size [8*SLP, 64] DRAM accumulator at the
#       global dst row.  Self-loops ride along as ordinary (v, v) edges.
#     * one ReduceScatter (add) hands each core the reduced rows of its own
#       slice — far cheaper than all-gathering the full table since collective
#       cost tracks the OUTPUT size.
#   BatchNorm stats via an accumulated A^T[A|1] matmul + algebraic reduction
#   + one tiny [64,65] all-reduce (overlapped with the W1 matmul work).
#   Layer 2 + pooling collapse into dense matmuls: since mean-pool
#   P (and the outer D^-1/2) are linear, out = sum_k (R_k @ xs2_k) W2 + b2
#   with R_k[g, u] = sum_{edges u->w owned by k} P[g,w] dinv_w  (+ self term),
#   built on the host from pure index data.  No second edge phase, no second
#   table, no second big collective — just 50 accumulating [128,64]x[128,64]
#   matmuls and a tiny [64,64] all-reduce.
#
# dma_scatter_add races (loses updates) for duplicate dst rows within one
# instruction, so edges are packed into instruction "bins" with unique dst
# rows per bin via rotation binning: occurrence o of dst row d goes to bin
# (d + o) % nbins.  Scatter row indices are signed int16, so bins are split
# into lo (row < 32768) / hi buckets scattered at different out_ap bases.
# Pad slots gather row 0 and scatter into a dead pad row (junk, multiplied by
# dinv=0 downstream).
#
# Host-side preprocessing uses only index data (edge_index, batch): degree
# computation, edge partitioning/binning, the R_k pooling matrices.  Feature
# data is never touched on the host.

import os

import numpy as np

N_NODES = 50000
N_EDGES = 800000
D = 64
NCORES = 8
NUM_GRAPHS = 64
BN_EPS = 1e-5
SPLIT = 32768  # int16 scatter index limit


class Cfg:
    def __init__(self, n, sl):
        self.N = n                    # total nodes
        self.SL = sl                  # owned nodes per core
        self.SLP = ((sl + 127) // 128) * 128   # padded slice rows
        assert self.SL < self.SLP, "need a dead pad row per slice"
        self.NT = self.SLP // 128     # 128-row node tiles per slice
        self.NG = NCORES * self.SLP   # padded global accumulator rows
        self.CAP = 7680               # max slots per gather/scatter instruction
        # per-instruction (bucket, slot count); filled by prepare_inputs
        self.seg = []                 # list of (bucket 0/1, padded slot count)


LAST_EXEC_TIME_NS = None
_NC_CACHE = {}
_LAST_IN_MAPS = None


def build(cfg):
    import concourse.mybir as mybir
    import concourse.tile as tile
    from concourse import bacc
    from concourse.masks import make_identity

    f32 = mybir.dt.float32
    i16 = mybir.dt.int16
    SL, SLP, NT, NG = cfg.SL, cfg.SLP, cfg.NT, cfg.NG
    NN = float(cfg.N)
    RG = [list(range(NCORES))]
    segs = cfg.seg
    tot_s = sum(c for _, c in segs)

    nc = bacc.Bacc(
        "TRN2", target_bir_lowering=False, debug=False, num_devices=NCORES
    )

    # --- external inputs (per-core values supplied via in_maps) ---
    xsl = nc.declare_dram_parameter("xsl", [SLP, D], f32, isOutput=False)
    dinv_in = nc.declare_dram_parameter("dinv_in", [128, NT], f32, isOutput=False)
    gidx_d = nc.declare_dram_parameter("gidx", [128, tot_s // 16], i16, isOutput=False)
    sidx_d = nc.declare_dram_parameter("sidx", [128, tot_s // 16], i16, isOutput=False)
    rkt_d = nc.declare_dram_parameter("rkt", [SLP, D], f32, isOutput=False)
    p1_d = nc.declare_dram_parameter("p1", [1, NUM_GRAPHS], f32, isOutput=False)
    w1_d = nc.declare_dram_parameter("w1", [D, D], f32, isOutput=False)
    b1_d = nc.declare_dram_parameter("b1", [D, 1], f32, isOutput=False)
    ga_d = nc.declare_dram_parameter("ga", [D, 1], f32, isOutput=False)
    be_d = nc.declare_dram_parameter("be", [D, 1], f32, isOutput=False)
    w2_d = nc.declare_dram_parameter("w2", [D, D], f32, isOutput=False)
    b2_d = nc.declare_dram_parameter("b2", [1, D], f32, isOutput=False)
    out_d = nc.declare_dram_parameter("out", [NUM_GRAPHS, D], f32, isOutput=True)

    # --- internal DRAM ---
    table1 = nc.dram_tensor("table1", [SLP, D], f32)
    acc = nc.dram_tensor("acc", [NG, D], f32)
    rs_out = nc.dram_tensor("rs_out", [SLP, D], f32)
    ars_in = nc.dram_tensor("ars_in", [D, D + 1], f32)
    ars_out = nc.dram_tensor("ars_out", [D, D + 1], f32, addr_space="Shared")
    aro_in = nc.dram_tensor("aro_in", [NUM_GRAPHS, D], f32)
    aro_out = nc.dram_tensor("aro_out", [NUM_GRAPHS, D], f32, addr_space="Shared")

    with tile.TileContext(nc) as tc:
        with (
            tc.tile_pool(name="const", bufs=1) as const,
            tc.tile_pool(name="persist", bufs=1) as persist,
            tc.tile_pool(name="work", bufs=2) as work,
            tc.tile_pool(name="msgp", bufs=3) as msgp,
            tc.tile_pool(name="spsum", bufs=1, space="PSUM") as spsum,
            tc.tile_pool(name="wpsum", bufs=2, space="PSUM") as wpsum,
        ):
            ablate = os.environ.get("GNN_ABLATE", "")

            # --- zero the DRAM accumulator (4 chunks on idle engines) ---
            zt = persist.tile([128, 6400], f32, name="zt")
            nc.vector.memset(zt[:], 0.0)
            acc_flat = acc[:, :].rearrange("n d -> (n d)")
            CHUNK = NG * D // 4
            for i, eng in enumerate((nc.scalar, nc.scalar, nc.sync, nc.sync)):
                ap = acc_flat.rearrange("(c p x) -> c p x", c=4, p=128)[i]
                eng.dma_start(out=ap, in_=zt[:, : CHUNK // 128])

            # --- constants into SBUF ---
            w1s = const.tile([D, D], f32)
            nc.sync.dma_start(out=w1s[:], in_=w1_d[:, :])
            w2s = const.tile([D, D], f32)
            nc.sync.dma_start(out=w2s[:], in_=w2_d[:, :])
            b1c = const.tile([D, 1], f32)
            nc.sync.dma_start(out=b1c[:], in_=b1_d[:, :])
            gac = const.tile([D, 1], f32)
            nc.sync.dma_start(out=gac[:], in_=ga_d[:, :])
            bec = const.tile([D, 1], f32)
            nc.sync.dma_start(out=bec[:], in_=be_d[:, :])
            b2r = const.tile([1, D], f32)
            nc.sync.dma_start(out=b2r[:], in_=b2_d[:, :])
            p1s = const.tile([1, NUM_GRAPHS], f32)
            nc.sync.dma_start(out=p1s[:], in_=p1_d[:, :])
            dinvs = const.tile([128, NT], f32)
            nc.sync.dma_start(out=dinvs[:], in_=dinv_in[:, :])
            ident = const.tile([128, 128], f32)
            make_identity(nc, ident[:])
            ones64 = const.tile([D, 1], f32)
            nc.vector.memset(ones64[:], 1.0)
            epsc = const.tile([D, 1], f32)
            nc.vector.memset(epsc[:], BN_EPS)

            # --- edge index tiles ---
            gidx_t = persist.tile([128, tot_s // 16], i16, name="gidx_t")
            nc.scalar.dma_start(out=gidx_t[:], in_=gidx_d[:, :])
            sidx_t = persist.tile([128, tot_s // 16], i16, name="sidx_t")
            nc.scalar.dma_start(out=sidx_t[:], in_=sidx_d[:, :])

            # --- phase A: xs = dinv * x -> local gather table ---
            xs_t = work.tile([128, NT, D], f32, tag="big", name="xs_t")
            nc.sync.dma_start(
                out=xs_t[:], in_=xsl[:, :].rearrange("(g p) d -> p g d", p=128)
            )
            dinv_b = dinvs[:, :].rearrange("p (g o) -> p g o", o=1).to_broadcast(
                [128, NT, D]
            )
            nc.vector.tensor_tensor(
                out=xs_t[:], in0=xs_t[:], in1=dinv_b, op=mybir.AluOpType.mult
            )
            nc.sync.dma_start(
                out=table1[:, :].rearrange("(g p) d -> p g d", p=128), in_=xs_t[:]
            )

            # --- R_k^T for layer 2 (loaded during the edge phase) ---
            rkt_t = persist.tile([128, NT, D], f32, name="rkt_t")
            nc.sync.dma_start(
                out=rkt_t[:], in_=rkt_d[:, :].rearrange("(g p) d -> p g d", p=128)
            )

            # --- phase B: layer-1 edges (gather from local table, scatter-add
            #     into the global accumulator) ---
            if "noedge" not in ablate:
                off = 0
                for bkt, cnt in segs:
                    msg = msgp.tile([128, cfg.CAP // 128, D], f32, tag="msg",
                                    name="msg")
                    nc.gpsimd.dma_gather(
                        out_ap=msg[:, : cnt // 128, :],
                        in_ap=table1[0:SLP, :],
                        idxs_ap=gidx_t[:, off : off + cnt // 16],
                        num_idxs=cnt, num_idxs_reg=cnt, elem_size=D,
                        single_packet=False, queue_num=0,
                    )
                    base = 0 if bkt == 0 else SPLIT
                    span = SPLIT if bkt == 0 else NG - SPLIT
                    nc.gpsimd.dma_scatter_add(
                        acc[base : base + span, :],
                        msg[:, : cnt // 128, :],
                        sidx_t[:, off : off + cnt // 16],
                        cnt, cnt, D,
                        single_packet=False, queue_num=0,
                    )
                    off += cnt // 16

            # --- phase C: ReduceScatter -> own reduced slice ---
            def do_cc(kind, op, ins_ap, outs_ap):
                if "nocc" in ablate:
                    nc.sync.dma_start(
                        out=outs_ap, in_=ins_ap[0 : outs_ap.shape[0], :]
                    )
                    return
                nc.gpsimd.collective_compute(
                    kind, op, replica_groups=RG, ins=[ins_ap], outs=[outs_ap],
                )

            do_cc("ReduceScatter", mybir.AluOpType.add, acc[:, :], rs_out[:, :])

            # --- phase D: dense layer-1 + BN stats ---
            import concourse.mybir as mb

            z_t = work.tile([128, NT, D], f32, tag="big", name="z_t")
            nc.sync.dma_start(
                out=z_t[:], in_=rs_out[:, :].rearrange("(g p) d -> p g d", p=128)
            )
            aggs = persist.tile([128, NT, D + 1], f32, name="aggs")
            nc.vector.memset(aggs[:, :, D : D + 1], 1.0)
            nc.vector.tensor_tensor(
                out=aggs[:, :, :D], in0=z_t[:], in1=dinv_b, op=mybir.AluOpType.mult
            )

            stats_ps = spsum.tile([D, D + 1], f32, name="stats_ps")
            hT_big = persist.tile([D, NT * 128], f32, name="hT_big")
            ND = NT if "noD" not in ablate else 1
            for b in range(ND):
                nc.tensor.matmul(
                    out=stats_ps[:], lhsT=aggs[:, b, :D], rhs=aggs[:, b, :],
                    start=(b == 0), stop=(b == ND - 1),
                )
            # stats all-reduce launched before the transposes/W1 matmuls so the
            # collective overlaps with PE work
            stats_sb = persist.tile([D, D + 1], f32, name="stats_sb")
            nc.vector.tensor_copy(out=stats_sb[:], in_=stats_ps[:])
            nc.sync.dma_start(out=ars_in[:, :], in_=stats_sb[:])
            do_cc("AllReduce", mybir.AluOpType.add, ars_in[:, :], ars_out[:, :])

            for b0 in range(0, ND, 4):
                bn = min(4, ND - b0)
                tp_ps = wpsum.tile([D, 512], f32, tag="ps_a", name="tp_ps")
                for j in range(bn):
                    b = b0 + j
                    nc.tensor.transpose(
                        out=tp_ps[:, j * 128 : (j + 1) * 128],
                        in_=aggs[:, b, :D], identity=ident[:],
                    )
                aggsT = work.tile([D, 512], f32, tag="aggsT", name="aggsT", bufs=2)
                nc.vector.tensor_copy(out=aggsT[:, : bn * 128], in_=tp_ps[:, : bn * 128])
                hT_ps = wpsum.tile([D, 512], f32, tag="ps_b", name="hT_ps")
                nc.tensor.matmul(
                    out=hT_ps[:, : bn * 128], lhsT=w1s[:], rhs=aggsT[:, : bn * 128],
                    start=True, stop=True,
                )
                nc.vector.tensor_copy(
                    out=hT_big[:, b0 * 128 : (b0 + bn) * 128],
                    in_=hT_ps[:, : bn * 128],
                )

            st = persist.tile([D, D + 1], f32, name="st")
            nc.sync.dma_start(out=st[:], in_=ars_out[:, :])

            # --- phase E: BN scalar algebra ---
            q_ps = wpsum.tile([D, 1], f32, tag="ps_a", name="q_ps")
            nc.tensor.matmul(out=q_ps[:], lhsT=w1s[:], rhs=st[:, D : D + 1], start=True, stop=True)
            mu = persist.tile([D, 1], f32, name="mu")
            nc.vector.tensor_scalar(
                out=mu[:], in0=q_ps[:], scalar1=1.0 / NN, scalar2=b1c[:],
                op0=mybir.AluOpType.mult, op1=mybir.AluOpType.add,
            )
            t1_ps = wpsum.tile([D, D], f32, tag="ps_b", name="t1_ps")
            nc.tensor.matmul(out=t1_ps[:], lhsT=st[:, :D], rhs=w1s[:], start=True, stop=True)
            m_sb = work.tile([D, D], f32, tag="m_sb", name="m_sb")
            nc.vector.tensor_tensor(out=m_sb[:], in0=w1s[:], in1=t1_ps[:], op=mybir.AluOpType.mult)
            d_ps = wpsum.tile([D, 1], f32, tag="ps_b", name="d_ps")
            nc.tensor.matmul(out=d_ps[:], lhsT=m_sb[:], rhs=ones64[:], start=True, stop=True)

            var = persist.tile([D, 1], f32, name="var")
            nc.vector.tensor_scalar_mul(out=var[:], in0=d_ps[:], scalar1=1.0 / NN)
            t2 = work.tile([D, 1], f32, tag="t2", name="t2")
            nc.vector.tensor_scalar_mul(out=t2[:], in0=q_ps[:], scalar1=2.0 / NN)
            nc.vector.tensor_tensor(out=t2[:], in0=t2[:], in1=b1c[:], op=mybir.AluOpType.mult)
            nc.vector.tensor_tensor(out=var[:], in0=var[:], in1=t2[:], op=mybir.AluOpType.add)
            t3 = work.tile([D, 1], f32, tag="t3", name="t3")
            nc.vector.tensor_tensor(out=t3[:], in0=b1c[:], in1=b1c[:], op=mybir.AluOpType.mult)
            nc.vector.tensor_tensor(out=var[:], in0=var[:], in1=t3[:], op=mybir.AluOpType.add)
            t4 = work.tile([D, 1], f32, tag="t4", name="t4")
            nc.vector.tensor_tensor(out=t4[:], in0=mu[:], in1=mu[:], op=mybir.AluOpType.mult)
            nc.vector.tensor_tensor(out=var[:], in0=var[:], in1=t4[:], op=mybir.AluOpType.subtract)

            sd = work.tile([D, 1], f32, tag="sd", name="sd")
            nc.scalar.activation(sd[:], var[:], mb.ActivationFunctionType.Sqrt, bias=epsc[:])
            rstd = work.tile([D, 1], f32, tag="rstd", name="rstd")
            nc.vector.reciprocal(out=rstd[:], in_=sd[:])
            a_sb = persist.tile([D, 1], f32, name="a_sb")
            nc.vector.tensor_tensor(out=a_sb[:], in0=gac[:], in1=rstd[:], op=mybir.AluOpType.mult)
            c_sb = persist.tile([D, 1], f32, name="c_sb")
            t5 = work.tile([D, 1], f32, tag="t5", name="t5")
            nc.vector.tensor_tensor(out=t5[:], in0=mu[:], in1=a_sb[:], op=mybir.AluOpType.mult)
            nc.vector.tensor_tensor(out=c_sb[:], in0=bec[:], in1=t5[:], op=mybir.AluOpType.subtract)
            # hT tiles exclude the b1 bias; fold it into the BN offset:
            # relu(a*(h+b1) + c) = relu(a*h + (c + a*b1))
            t6 = work.tile([D, 1], f32, tag="t6", name="t6")
            nc.vector.tensor_tensor(out=t6[:], in0=a_sb[:], in1=b1c[:], op=mybir.AluOpType.mult)
            nc.vector.tensor_tensor(out=c_sb[:], in0=c_sb[:], in1=t6[:], op=mybir.AluOpType.add)

            # --- phase F: BN+ReLU, transpose back, dinv fold -> xs2 ---
            xs2 = persist.tile([128, NT, D], f32, name="xs2")
            NF = NT if "noF" not in ablate else 0
            for b0 in range(0, NF, 4):
                bn = min(4, NF - b0)
                h1T = work.tile([D, 512], f32, tag="h1T", name="h1T", bufs=2)
                nc.scalar.activation(
                    h1T[:, : bn * 128],
                    hT_big[:, b0 * 128 : (b0 + bn) * 128],
                    mb.ActivationFunctionType.Relu,
                    bias=c_sb[:], scale=a_sb[:],
                )
                for j in range(bn):
                    b = b0 + j
                    nm_ps = wpsum.tile([128, D], f32, tag="ps_a", name="nm_ps")
                    nc.tensor.transpose(
                        out=nm_ps[:], in_=h1T[:, j * 128 : (j + 1) * 128],
                        identity=ident[:D, :D],
                    )
                    nc.vector.tensor_scalar_mul(
                        out=xs2[:, b, :], in0=nm_ps[:], scalar1=dinvs[:, b : b + 1]
                    )

            # --- phase G: layer 2 + pooling as one accumulated matmul ---
            out_ps = spsum.tile([NUM_GRAPHS, D], f32, name="out_ps")
            NG2 = NT if "noG" not in ablate else 1
            for b in range(NG2):
                nc.tensor.matmul(
                    out=out_ps[:], lhsT=rkt_t[:, b, :], rhs=xs2[:, b, :],
                    start=(b == 0), stop=False,
                )
            nc.tensor.matmul(out=out_ps[:], lhsT=p1s[:], rhs=b2r[:], start=False, stop=True)
            out_sb = persist.tile([NUM_GRAPHS, D], f32, name="out_sb")
            nc.vector.tensor_copy(out=out_sb[:], in_=out_ps[:])
            nc.sync.dma_start(out=aro_in[:, :], in_=out_sb[:])
            do_cc("AllReduce", mybir.AluOpType.add, aro_in[:, :], aro_out[:, :])
            nc.sync.dma_start(out=out_d[:, :], in_=aro_out[:, :])

    nc.compile()
    return nc


def _wrap16(v, n):
    """idx j at [j%16, j//16], replicated to 128 partitions (8 Q7 cores)."""
    assert v.shape[0] == n and n % 16 == 0
    t = v.astype(np.int16).reshape(n // 16, 16).T
    return np.tile(t, (8, 1))


def _bin_edges(gsrc, gdst, nbins):
    """Rotation binning: occurrence o of dst row d -> bin (d + o) % nbins.
    Rows are unique within each bin as long as multiplicity <= nbins.
    Returns per-bin (src, dst) arrays."""
    order = np.argsort(gdst, kind="stable")
    sd, ss = gdst[order], gsrc[order]
    if sd.shape[0] == 0:
        return [(np.zeros(0, np.int64), np.zeros(0, np.int64))] * nbins
    change = np.r_[True, sd[1:] != sd[:-1]]
    starts = np.flatnonzero(change)
    gid = np.cumsum(change) - 1
    occ = np.arange(sd.shape[0]) - starts[gid]
    assert int(occ.max()) < nbins, (int(occ.max()), nbins)
    b = (sd + occ) % nbins
    out = []
    for i in range(nbins):
        m = b == i
        out.append((ss[m], sd[m]))
    return out


def prepare_inputs(cfg, x, edge_index, batch, W1, b1, gamma, beta, W2, b2):
    """Host-side index preprocessing + per-core input maps.  Fills cfg.seg."""
    SL, SLP, NG = cfg.SL, cfg.SLP, cfg.NG
    n = cfg.N

    x = np.ascontiguousarray(np.asarray(x, dtype=np.float32))
    src = np.asarray(edge_index[0], dtype=np.int64)
    dst = np.asarray(edge_index[1], dtype=np.int64)
    batch = np.asarray(batch, dtype=np.int64)
    W1 = np.asarray(W1, dtype=np.float32)
    b1 = np.asarray(b1, dtype=np.float32)
    gamma = np.asarray(gamma, dtype=np.float32)
    beta = np.asarray(beta, dtype=np.float32)
    W2 = np.asarray(W2, dtype=np.float32)
    b2 = np.asarray(b2, dtype=np.float32)

    deg = np.bincount(dst, minlength=n).astype(np.float32) + 1.0  # + self-loop
    dinv = (1.0 / np.sqrt(deg)).astype(np.float32)

    cnt = np.bincount(batch, minlength=NUM_GRAPHS).astype(np.float32)
    w_graph = 1.0 / np.maximum(cnt, 1.0)
    pd = w_graph[batch] * dinv          # P[batch[v], v] * dinv_v  per node

    owner = src // SL
    src_local = src - owner * SL
    gdst = (dst // SL) * SLP + (dst - (dst // SL) * SL)
    loops = np.arange(n, dtype=np.int64)
    l_owner = loops // SL
    l_local = loops - l_owner * SL
    l_gdst = l_owner * SLP + l_local

    # per-core edge lists (edges by src owner + own self-loops), lo/hi buckets
    per_core = []
    for k in range(NCORES):
        sel = owner == k
        ls = l_owner == k
        es = np.concatenate([src_local[sel], l_local[ls]])
        ed = np.concatenate([gdst[sel], l_gdst[ls]])
        lo = ed < SPLIT
        per_core.append(((es[lo], ed[lo]), (es[~lo], ed[~lo])))

    # choose bin counts (shared across cores) per bucket
    def max_mult(arrs):
        m = 1
        for a in arrs:
            if a.shape[0]:
                m = max(m, int(np.bincount(a).max()))
        return m

    seg, core_bins = [], [[] for _ in range(NCORES)]
    for bkt in (0, 1):
        counts = [per_core[k][bkt][0].shape[0] for k in range(NCORES)]
        mm = max_mult([per_core[k][bkt][1] for k in range(NCORES)])
        nbins = max((max(counts) + cfg.CAP - 200) // (cfg.CAP - 200), mm, 1)
        while True:
            allb = [
                _bin_edges(per_core[k][bkt][0], per_core[k][bkt][1], nbins)
                for k in range(NCORES)
            ]
            sizes = [
                ((max(allb[k][i][0].shape[0] for k in range(NCORES)) + 127)
                 // 128) * 128
                for i in range(nbins)
            ]
            if all(s <= cfg.CAP for s in sizes):
                break
            nbins += 1
        for i in range(nbins):
            if sizes[i] == 0:
                continue
            seg.append((bkt, sizes[i]))
            for k in range(NCORES):
                core_bins[k].append((bkt, sizes[i], allb[k][i]))

    cfg.seg = seg

    # dead pad rows for scatter padding (always zero * dinv=0 downstream)
    trash_lo = SL                      # core 0's first pad row, < SPLIT
    trash_hi = NG - (SLP - SL)         # core 7's first pad row, >= SPLIT
    assert trash_hi >= SPLIT

    in_maps = []
    for k in range(NCORES):
        gl_parts, sc_parts = [], []
        for bkt, size, (es, ed) in core_bins[k]:
            m = es.shape[0]
            g = np.zeros(size, dtype=np.int64)
            s = np.full(size, (trash_lo if bkt == 0 else trash_hi - SPLIT),
                        dtype=np.int64)
            order = np.argsort(es, kind="stable")  # src-sorted for locality
            g[:m] = es[order]
            s[:m] = ed[order] - (0 if bkt == 0 else SPLIT)
            gl_parts.append(_wrap16(g, size))
            sc_parts.append(_wrap16(s, size))
        gidx = np.concatenate(gl_parts, axis=1)
        sidx = np.concatenate(sc_parts, axis=1)

        lo, hi = k * SL, min((k + 1) * SL, n)
        nsl = hi - lo
        xsl = np.zeros((SLP, D), dtype=np.float32)
        xsl[:nsl] = x[lo:hi]
        dsl = np.zeros(SLP, dtype=np.float32)
        dsl[:nsl] = dinv[lo:hi]
        dinv_in = dsl.reshape(cfg.NT, 128).T.copy()

        # R_k^T [SLP, 64]: R_kT[u, g] = sum_{edges (k*SL+u) -> w} P[g,w]*dinv_w
        #                             + P[g, k*SL+u]*dinv_{k*SL+u}
        sel = owner == k
        rkt = np.zeros((SLP, NUM_GRAPHS), dtype=np.float32)
        np.add.at(rkt, (src_local[sel], batch[dst[sel]]), pd[dst[sel]])
        rkt[np.arange(nsl), batch[lo:hi]] += pd[lo:hi]

        p1 = np.zeros((1, NUM_GRAPHS), dtype=np.float32)
        np.add.at(p1[0], batch[lo:hi], w_graph[batch[lo:hi]])

        in_maps.append({
            "xsl": xsl,
            "dinv_in": dinv_in,
            "gidx": np.ascontiguousarray(gidx),
            "sidx": np.ascontiguousarray(sidx),
            "rkt": rkt,
            "p1": p1,
            "w1": W1,
            "b1": b1.reshape(D, 1),
            "ga": gamma.reshape(D, 1),
            "be": beta.reshape(D, 1),
            "w2": W2,
            "b2": b2.reshape(1, D),
        })
    return in_maps


def kernel(x, edge_index, batch, W1, b1, gamma, beta, W2, b2):
    global LAST_EXEC_TIME_NS
    from concourse.bass_utils import run_bass_kernel_spmd

    cfg = Cfg(N_NODES, N_NODES // NCORES)
    in_maps = prepare_inputs(cfg, x, edge_index, batch, W1, b1, gamma, beta, W2, b2)

    key = (cfg.N, cfg.SL, tuple(cfg.seg))
    if key not in _NC_CACHE:
        _NC_CACHE[key] = build(cfg)
    nc = _NC_CACHE[key]
    global _LAST_IN_MAPS
    _LAST_IN_MAPS = in_maps

    trace = bool(int(os.environ.get("BASS_GNN_TRACE", "0")))
    if trace:
        try:
            res = run_bass_kernel_spmd(nc, in_maps, list(range(NCORES)), trace=True)
        except Exception:
            res = run_bass_kernel_spmd(nc, in_maps, list(range(NCORES)), trace=False)
    else:
        res = run_bass_kernel_spmd(nc, in_maps, list(range(NCORES)), trace=False)
    LAST_EXEC_TIME_NS = res.exec_time_ns
    return np.asarray(res.results[0]["out"], dtype=np.float32)


def modeled_time_ns(x=None, edge_index=None, **kw):
    """Cost-model execution time (MultiCoreSim, mocked collectives) for the
    current cached program; used when NTFF tracing is unavailable."""
    if not _NC_CACHE:
        return None
    nc = next(iter(_NC_CACHE.values()))
    ins = _LAST_IN_MAPS
    if ins is None:
        return None
    from concourse.bass_interp import MultiCoreSim

    sim = MultiCoreSim(nc, 2, debug_mock_collectives_without_correctness=True)
    for i, core in sim.cores.items():
        for name, val in ins[i].items():
            core.tensor(name)[:] = val
    sim.simulate()
    return int(sim.global_time)
